# revision 23
# baseline (speedup 1.0000x reference)
"""Trainium2 Bass kernel for cross "efficient attention".

Reference computation (per batch b, head h, with C=128, HEADS=8, hc=16, n=16384):
    k = x2[b].reshape(HEADS, hc, n); v = x1[b].reshape(HEADS, hc, n)
    key_sm   = softmax(k, axis=-1)          # over n
    query_sm = softmax(k, axis=1)           # over hc (head channels)
    context  = key_sm @ v^T                 # (hc, hc)
    out[b,h] = context^T @ query_sm         # (hc, n)

Sharding: data-parallel over batch B=8 across the 8 NeuronCores (no
collectives).  Inputs are cast to bf16 on the host (tolerance is 2e-2;
bf16 end-to-end measures ~4e-3), halving HBM traffic, and x1 is laid
out host-side as [128, N/128, C] blocks so every DMA descriptor is a
contiguous run >= 4 KiB.

Per-core pipeline (N = 16384 in 4 slabs of 4096 = 32 chunks of 128):
  pass 1 (per chunk j):
    MM_t : transpose-mode matmul -> te PSUM bf16 (grouped 8 chunks/tile)
    MM_cs: matmul(lhsT=exp_chunk, rhs=ind8) -> per-slab PSUM f32
           accumulator [128, 32*8] (per-head colsums, transposed layout)
    one wide vector copy te -> eT slab buffer per group (bf16->bf16 2x)
    MM_ctx (one slab lag, interleaved per group): ctx += eT^T @ vT
  per slab: one reciprocal_approx_fast [128,256] PSUM->SBUF (rcp_all)
  bd = (ctx / rowsum) * blockdiag_mask  (bf16 [C,C])
  pass 2 (per 2048 block, chunk j): MM_att: matmul(lhsT=exp_chunk,
    rhs=bd) -> attT PSUM f32 [128, 2048]; one vector tensor_mul per
    block with the per-head reciprocals broadcast via a stride-0 AP ->
    bf16 out tile -> DMA out (ACT HWDGE ring).
Output leaves the device transposed ([128, N/128, C] blocks); the host
reassembles [C, H, W].
"""

import numpy as np
from contextlib import ExitStack

B, C, H, W = 8, 128, 128, 128
N = H * W                 # 16384
HEADS, HC = 8, 16
NCORES = 8
# pass-1 slab widths: wide while DMA-bound, tapered at the end so the
# final slab's exp->transpose->copy->ctx chain (which gates pass 2) is
# short.
SLABS = [1024, 2048, 4096, 4096, 2048, 1024, 1024, 512, 512]
NSLAB = len(SLABS)
assert sum(SLABS) == N
NB = N // C               # 128 chunk-blocks total
GRP = 8                   # transpose chunks batched per PSUM group tile
OB = 2048                 # pass-2 output block width
NOB = N // OB             # 8
OCH = OB // C             # chunks per output block = 16

_cache: dict = {}


def _build():
    import concourse.bass as bass
    import concourse.tile as tile
    from concourse import bacc, mybir

    FP32 = mybir.dt.float32
    BF16 = mybir.dt.bfloat16
    AF = mybir.ActivationFunctionType

    nc = bacc.Bacc("TRN2", target_bir_lowering=False, debug=False)

    x1t_d = nc.dram_tensor("x1t", [C, NB, C], BF16, kind="ExternalInput")
    x2_d = nc.dram_tensor("x2", [C, N], BF16, kind="ExternalInput")
    id_d = nc.dram_tensor("ident", [C, C], BF16, kind="ExternalInput")
    ind8_d = nc.dram_tensor("ind8", [C, HEADS], BF16, kind="ExternalInput")
    bd8_d = nc.dram_tensor("bd8", [C, C], BF16, kind="ExternalInput")
    out_d = nc.dram_tensor("out", [C, NB, C], BF16, kind="ExternalOutput")

    with tile.TileContext(nc) as tc:
        with ExitStack() as ctx:
            persist = ctx.enter_context(tc.tile_pool(name="persist", bufs=1))
            x2ld = ctx.enter_context(tc.tile_pool(name="x2ld", bufs=6))
            vTp = ctx.enter_context(tc.tile_pool(name="vTp", bufs=6))
            eTp = ctx.enter_context(tc.tile_pool(name="eTp", bufs=3))
            outp = ctx.enter_context(tc.tile_pool(name="outp", bufs=3))
            qtmp = ctx.enter_context(tc.tile_pool(name="qtmp", bufs=2))
            smalls = ctx.enter_context(tc.tile_pool(name="smalls", bufs=1))

            # one exp tile per slab: a single big tile would serialize the
            # pipeline on write-after-read hazards (slab i+1's exp write
            # waits for slab i's transpose reads at tile granularity)
            exp_tiles = [
                persist.tile([C, SW], BF16, tag=f"exp{i}", name=f"exp{i}")
                for i, SW in enumerate(SLABS)
            ]
            rcp_all = persist.tile([C, NB * HEADS], FP32, tag="rcp_all")
            rs_acc = smalls.tile([C, NSLAB], FP32, tag="rs_acc")
            ident = smalls.tile([C, C], BF16, tag="ident")
            ind8 = smalls.tile([C, HEADS], BF16, tag="ind8")
            bd8 = smalls.tile([C, C], BF16, tag="bd8")
            bd = smalls.tile([C, C], BF16, tag="bd")

            with tc.tile_pool(name="psctx", bufs=1, space="PSUM") as ps_ctx, \
                 tc.tile_pool(name="pstre", bufs=4, space="PSUM") as ps_te, \
                 tc.tile_pool(name="pscs", bufs=2, space="PSUM") as ps_cs:
                ctx_ps = ps_ctx.tile([C, C], FP32, tag="ctx")

                mm_idx = 0
                pending = []   # (eT_ap, vT_ap) per not-yet-contracted chunk

                def emit_ctx(k):
                    # emit ctx matmuls for the first k pending chunks
                    nonlocal mm_idx
                    for eTc, vTc in pending[:k]:
                        nc.tensor.matmul(
                            ctx_ps[:], eTc, vTc,
                            start=(mm_idx == 0),
                            stop=(mm_idx == NB - 1),
                        )
                        mm_idx += 1
                    del pending[:k]

                off = 0
                chunk_aps = []   # global chunk index -> exp chunk AP
                for i, SW in enumerate(SLABS):
                    nch = SW // C
                    ngrp = (nch + GRP - 1) // GRP
                    x2t = x2ld.tile([C, SW], BF16, tag="x2t")
                    nc.sync.dma_start(out=x2t[:], in_=x2_d[:, bass.ds(off, SW)])
                    vT = vTp.tile([C, SW], BF16, tag="vT")
                    nc.sync.dma_start(
                        out=vT[:].rearrange("p (j c) -> p j c", c=C),
                        in_=x1t_d[:, bass.ds(off // C, nch), :],
                    )
                    if i == 0:
                        # constants ride the ACT ring so they don't delay
                        # the slab loads on the SP ring
                        nc.scalar.dma_start(out=ident[:], in_=id_d[:])
                        nc.scalar.dma_start(out=ind8[:], in_=ind8_d[:])
                        nc.scalar.dma_start(out=bd8[:], in_=bd8_d[:])

                    exp_sl = exp_tiles[i]
                    nc.scalar.activation(
                        exp_sl[:], x2t[:], AF.Exp,
                        accum_out=rs_acc[:, i:i + 1],
                    )

                    # per-chunk transpose (bf16, grouped PSUM tiles) +
                    # colsum (f32 accum tile); one wide copy per group;
                    # pending (prev-slab) ctx matmuls interleaved per group
                    eT = eTp.tile([C, SW], BF16, tag="eT")
                    eTv = eT[:].rearrange("p (j c) -> p j c", c=C)
                    vTv = vT[:].rearrange("p (j c) -> p j c", c=C)
                    cs_ps = ps_cs.tile([C, nch * HEADS], FP32, tag="cs")
                    for g in range(ngrp):
                        gsz = min(GRP, nch - g * GRP)
                        te = ps_te.tile([C, gsz * C], BF16, tag="te")
                        fresh = []
                        for jj in range(gsz):
                            j = g * GRP + jj
                            e_chunk = exp_sl[:, bass.ds(j * C, C)]
                            chunk_aps.append(e_chunk)
                            nc.tensor.transpose(
                                te[:, bass.ds(jj * C, C)], e_chunk, ident[:]
                            )
                            nc.tensor.matmul(
                                cs_ps[:, bass.ds(j * HEADS, HEADS)],
                                e_chunk, ind8[:],
                            )
                            fresh.append((eTv[:, j, :], vTv[:, j, :]))
                        nc.vector.tensor_copy(
                            eT[:, bass.ds(g * GRP * C, gsz * C)], te[:]
                        )
                        # ctx matmuls lag one group behind the copies
                        emit_ctx(len(pending))
                        pending.extend(fresh)

                    # per-slab: one reciprocal over the colsum accumulator
                    nc.vector.reciprocal_approx_fast(
                        out=rcp_all[:, bass.ds(off // C * HEADS, nch * HEADS)],
                        in_=cs_ps[:],
                    )
                    off += SW
                emit_ctx(len(pending))

                # ---- block-diagonal context weights ----
                rowsum = smalls.tile([C, 1], FP32, tag="rowsum")
                nc.vector.tensor_reduce(
                    rowsum[:], rs_acc[:], mybir.AxisListType.X, mybir.AluOpType.add
                )
                rs_rcp = smalls.tile([C, 1], FP32, tag="rs_rcp")
                nc.vector.reciprocal(rs_rcp[:], rowsum[:])
                scaled = smalls.tile([C, C], BF16, tag="scaled")
                nc.vector.tensor_scalar(
                    scaled[:], ctx_ps[:], rs_rcp[:, 0:1], None, mybir.AluOpType.mult
                )
                nc.vector.tensor_mul(bd[:], scaled[:], bd8[:])

            # ---- pass 2: attended (transposed), normalize, store ----
            with tc.tile_pool(name="psatt", bufs=2, space="PSUM") as ps_att:
                for b in range(NOB):
                    att = ps_att.tile([C, OB], FP32, tag="att")
                    for j in range(OCH):
                        nc.tensor.matmul(
                            att[:, bass.ds(j * C, C)],
                            chunk_aps[b * OCH + j],
                            bd[:],
                        )
                    ot = outp.tile([C, OB], BF16, tag="ot")
                    rcpv = (
                        rcp_all[:, bass.ds(b * OCH * HEADS, OCH * HEADS)]
                        .rearrange("p (j h) -> p j h", h=HEADS)
                        .broadcast_to([C, OCH, HEADS, HC])
                    )
                    if b in (1, 4, 7):
                        # normalize off the critical vector path: scalar
                        # casts PSUM->SBUF, gpsimd does the multiply
                        qt = qtmp.tile([C, OB], BF16, tag="qt")
                        nc.scalar.copy(qt[:], att[:])
                        nc.gpsimd.tensor_mul(
                            ot[:].rearrange("p (j h c) -> p j h c", h=HEADS, c=HC),
                            qt[:].rearrange("p (j h c) -> p j h c", h=HEADS, c=HC),
                            rcpv,
                        )
                    else:
                        nc.vector.tensor_mul(
                            ot[:].rearrange("p (j h c) -> p j h c", h=HEADS, c=HC),
                            att[:].rearrange("p (j h c) -> p j h c", h=HEADS, c=HC),
                            rcpv,
                        )
                    eng = nc.scalar if b % 2 == 0 else nc.sync
                    eng.dma_start(
                        out=out_d[:, bass.ds(b * OCH, OCH), :],
                        in_=ot[:].rearrange("p (j c) -> p j c", c=C),
                    )

    nc.compile()
    return nc


def _get_nc():
    if "nc" not in _cache:
        _cache["nc"] = _build()
    return _cache["nc"]


def _consts_np():
    import ml_dtypes

    bf16 = ml_dtypes.bfloat16
    ident = np.eye(C, dtype=np.float32).astype(bf16)
    ind8 = np.zeros((C, HEADS), dtype=np.float32)
    for h in range(HEADS):
        ind8[h * HC:(h + 1) * HC, h] = 1.0
    bd8 = np.zeros((C, C), dtype=np.float32)
    for h in range(HEADS):
        bd8[h * HC:(h + 1) * HC, h * HC:(h + 1) * HC] = 1.0
    return ident, ind8.astype(bf16), bd8.astype(bf16)


def _to_np(a) -> np.ndarray:
    """Materialize to float32 numpy; retry once on a transient bad fetch
    (device-backed arrays have been observed to materialize NaNs once)."""
    out = np.asarray(a, dtype=np.float32)
    if np.isnan(out).any():
        out = np.asarray(a, dtype=np.float32)
    return out


def make_in_maps(x1: np.ndarray, x2: np.ndarray):
    import ml_dtypes

    bf16 = ml_dtypes.bfloat16
    x1 = _to_np(x1).reshape(B, C, N)
    x2 = _to_np(x2).reshape(B, C, N)
    # x1 blocked-transposed: x1t[b, p, j, c] = x1[b, c, j*128 + p]
    x1t = np.ascontiguousarray(
        x1.reshape(B, C, NB, C).transpose(0, 3, 2, 1)
    ).astype(bf16)
    x2b = x2.astype(bf16)
    ident, ind8, bd8 = _consts_np()
    return [
        {"x1t": x1t[i], "x2": x2b[i], "ident": ident, "ind8": ind8, "bd8": bd8}
        for i in range(NCORES)
    ]


def kernel(x1: np.ndarray, x2: np.ndarray) -> np.ndarray:
    from concourse.bass_utils import run_bass_kernel_spmd

    nc = _get_nc()
    in_maps = make_in_maps(x1, x2)
    res = run_bass_kernel_spmd(nc, in_maps, core_ids=list(range(NCORES)))
    outs = []
    for i in range(NCORES):
        o = np.asarray(res.results[i]["out"], dtype=np.float32)  # [128, NB, C]
        outs.append(o.transpose(2, 1, 0).reshape(C, N))          # [C, N]
    return np.stack(outs, axis=0).reshape(B, C, H, W)


# revision 24
# speedup vs baseline: 1.0385x; 1.0385x over previous
"""Trainium2 Bass kernel for cross "efficient attention".

Reference computation (per batch b, head h, with C=128, HEADS=8, hc=16, n=16384):
    k = x2[b].reshape(HEADS, hc, n); v = x1[b].reshape(HEADS, hc, n)
    key_sm   = softmax(k, axis=-1)          # over n
    query_sm = softmax(k, axis=1)           # over hc (head channels)
    context  = key_sm @ v^T                 # (hc, hc)
    out[b,h] = context^T @ query_sm         # (hc, n)

Sharding: data-parallel over batch B=8 across the 8 NeuronCores (no
collectives).  Inputs are cast to bf16 on the host (tolerance is 2e-2;
bf16 end-to-end measures ~4e-3), halving HBM traffic, and x1 is laid
out host-side as [128, N/128, C] blocks so every DMA descriptor is a
contiguous run >= 4 KiB.

Per-core pipeline (N = 16384 in 4 slabs of 4096 = 32 chunks of 128):
  pass 1 (per chunk j):
    MM_t : transpose-mode matmul -> te PSUM bf16 (grouped 8 chunks/tile)
    MM_cs: matmul(lhsT=exp_chunk, rhs=ind8) -> per-slab PSUM f32
           accumulator [128, 32*8] (per-head colsums, transposed layout)
    one wide vector copy te -> eT slab buffer per group (bf16->bf16 2x)
    MM_ctx (one slab lag, interleaved per group): ctx += eT^T @ vT
  per slab: one reciprocal_approx_fast [128,256] PSUM->SBUF (rcp_all)
  bd = (ctx / rowsum) * blockdiag_mask  (bf16 [C,C])
  pass 2 (per 2048 block, chunk j): MM_att: matmul(lhsT=exp_chunk,
    rhs=bd) -> attT PSUM f32 [128, 2048]; one vector tensor_mul per
    block with the per-head reciprocals broadcast via a stride-0 AP ->
    bf16 out tile -> DMA out (ACT HWDGE ring).
Output leaves the device transposed ([128, N/128, C] blocks); the host
reassembles [C, H, W].
"""

import numpy as np
from contextlib import ExitStack

B, C, H, W = 8, 128, 128, 128
N = H * W                 # 16384
HEADS, HC = 8, 16
NCORES = 8
# pass-1 slab widths: wide while DMA-bound, tapered at the end so the
# final slab's exp->transpose->copy->ctx chain (which gates pass 2) is
# short.
SLABS = [1024, 2048, 4096, 4096, 2048, 1024, 1024, 512, 512]
NSLAB = len(SLABS)
assert sum(SLABS) == N
NB = N // C               # 128 chunk-blocks total
GRP = 8                   # transpose chunks batched per PSUM group tile
OB = 1024                 # pass-2 output block width
NOB = N // OB             # 16
OCH = OB // C             # chunks per output block = 8
GPB = (1, 4, 7, 10, 13)   # pass-2 blocks normalized via scalar+gpsimd

_cache: dict = {}


def _build():
    import concourse.bass as bass
    import concourse.tile as tile
    from concourse import bacc, mybir

    FP32 = mybir.dt.float32
    BF16 = mybir.dt.bfloat16
    AF = mybir.ActivationFunctionType

    nc = bacc.Bacc("TRN2", target_bir_lowering=False, debug=False)

    x1t_d = nc.dram_tensor("x1t", [C, NB, C], BF16, kind="ExternalInput")
    x2_d = nc.dram_tensor("x2", [C, N], BF16, kind="ExternalInput")
    id_d = nc.dram_tensor("ident", [C, C], BF16, kind="ExternalInput")
    ind8_d = nc.dram_tensor("ind8", [C, HEADS], BF16, kind="ExternalInput")
    bd8_d = nc.dram_tensor("bd8", [C, C], BF16, kind="ExternalInput")
    out_d = nc.dram_tensor("out", [C, NB, C], BF16, kind="ExternalOutput")

    with tile.TileContext(nc) as tc:
        with ExitStack() as ctx:
            persist = ctx.enter_context(tc.tile_pool(name="persist", bufs=1))
            x2ld = ctx.enter_context(tc.tile_pool(name="x2ld", bufs=6))
            vTp = ctx.enter_context(tc.tile_pool(name="vTp", bufs=6))
            eTp = ctx.enter_context(tc.tile_pool(name="eTp", bufs=3))
            outp = ctx.enter_context(tc.tile_pool(name="outp", bufs=4))
            qtmp = ctx.enter_context(tc.tile_pool(name="qtmp", bufs=2))
            smalls = ctx.enter_context(tc.tile_pool(name="smalls", bufs=1))

            # one exp tile per slab: a single big tile would serialize the
            # pipeline on write-after-read hazards (slab i+1's exp write
            # waits for slab i's transpose reads at tile granularity)
            exp_tiles = [
                persist.tile([C, SW], BF16, tag=f"exp{i}", name=f"exp{i}")
                for i, SW in enumerate(SLABS)
            ]
            rcp_all = persist.tile([C, NB * HEADS], FP32, tag="rcp_all")
            rs_acc = smalls.tile([C, NSLAB], FP32, tag="rs_acc")
            ident = smalls.tile([C, C], BF16, tag="ident")
            ind8 = smalls.tile([C, HEADS], BF16, tag="ind8")
            bd8 = smalls.tile([C, C], BF16, tag="bd8")
            bd = smalls.tile([C, C], BF16, tag="bd")

            with tc.tile_pool(name="psctx", bufs=1, space="PSUM") as ps_ctx, \
                 tc.tile_pool(name="pstre", bufs=4, space="PSUM") as ps_te, \
                 tc.tile_pool(name="pscs", bufs=2, space="PSUM") as ps_cs:
                ctx_ps = ps_ctx.tile([C, C], FP32, tag="ctx")

                mm_idx = 0
                pending = []   # (eT_ap, vT_ap) per not-yet-contracted chunk

                def emit_ctx(k):
                    # emit ctx matmuls for the first k pending chunks
                    nonlocal mm_idx
                    for eTc, vTc in pending[:k]:
                        nc.tensor.matmul(
                            ctx_ps[:], eTc, vTc,
                            start=(mm_idx == 0),
                            stop=(mm_idx == NB - 1),
                        )
                        mm_idx += 1
                    del pending[:k]

                off = 0
                chunk_aps = []   # global chunk index -> exp chunk AP
                for i, SW in enumerate(SLABS):
                    nch = SW // C
                    ngrp = (nch + GRP - 1) // GRP
                    x2t = x2ld.tile([C, SW], BF16, tag="x2t")
                    nc.sync.dma_start(out=x2t[:], in_=x2_d[:, bass.ds(off, SW)])
                    vT = vTp.tile([C, SW], BF16, tag="vT")
                    nc.sync.dma_start(
                        out=vT[:].rearrange("p (j c) -> p j c", c=C),
                        in_=x1t_d[:, bass.ds(off // C, nch), :],
                    )
                    if i == 0:
                        # constants ride the ACT ring so they don't delay
                        # the slab loads on the SP ring
                        nc.scalar.dma_start(out=ident[:], in_=id_d[:])
                        nc.scalar.dma_start(out=ind8[:], in_=ind8_d[:])
                        nc.scalar.dma_start(out=bd8[:], in_=bd8_d[:])

                    exp_sl = exp_tiles[i]
                    nc.scalar.activation(
                        exp_sl[:], x2t[:], AF.Exp,
                        accum_out=rs_acc[:, i:i + 1],
                    )

                    # per-chunk transpose (bf16, grouped PSUM tiles) +
                    # colsum (f32 accum tile); one wide copy per group;
                    # pending (prev-slab) ctx matmuls interleaved per group
                    eT = eTp.tile([C, SW], BF16, tag="eT")
                    eTv = eT[:].rearrange("p (j c) -> p j c", c=C)
                    vTv = vT[:].rearrange("p (j c) -> p j c", c=C)
                    cs_ps = ps_cs.tile([C, nch * HEADS], FP32, tag="cs")
                    for g in range(ngrp):
                        gsz = min(GRP, nch - g * GRP)
                        te = ps_te.tile([C, gsz * C], BF16, tag="te")
                        fresh = []
                        for jj in range(gsz):
                            j = g * GRP + jj
                            e_chunk = exp_sl[:, bass.ds(j * C, C)]
                            chunk_aps.append(e_chunk)
                            nc.tensor.transpose(
                                te[:, bass.ds(jj * C, C)], e_chunk, ident[:]
                            )
                            nc.tensor.matmul(
                                cs_ps[:, bass.ds(j * HEADS, HEADS)],
                                e_chunk, ind8[:],
                            )
                            fresh.append((eTv[:, j, :], vTv[:, j, :]))
                        nc.vector.tensor_copy(
                            eT[:, bass.ds(g * GRP * C, gsz * C)], te[:]
                        )
                        # ctx matmuls lag one group behind the copies
                        emit_ctx(len(pending))
                        pending.extend(fresh)

                    # per-slab: one reciprocal over the colsum accumulator
                    nc.vector.reciprocal_approx_fast(
                        out=rcp_all[:, bass.ds(off // C * HEADS, nch * HEADS)],
                        in_=cs_ps[:],
                    )
                    off += SW
                emit_ctx(len(pending))

                # ---- block-diagonal context weights ----
                rowsum = smalls.tile([C, 1], FP32, tag="rowsum")
                nc.vector.tensor_reduce(
                    rowsum[:], rs_acc[:], mybir.AxisListType.X, mybir.AluOpType.add
                )
                rs_rcp = smalls.tile([C, 1], FP32, tag="rs_rcp")
                nc.vector.reciprocal(rs_rcp[:], rowsum[:])
                scaled = smalls.tile([C, C], BF16, tag="scaled")
                nc.vector.tensor_scalar(
                    scaled[:], ctx_ps[:], rs_rcp[:, 0:1], None, mybir.AluOpType.mult
                )
                nc.vector.tensor_mul(bd[:], scaled[:], bd8[:])

            # ---- pass 2: attended (transposed), normalize, store ----
            with tc.tile_pool(name="psatt", bufs=4, space="PSUM") as ps_att:
                for b in range(NOB):
                    att = ps_att.tile([C, OB], FP32, tag="att")
                    for j in range(OCH):
                        nc.tensor.matmul(
                            att[:, bass.ds(j * C, C)],
                            chunk_aps[b * OCH + j],
                            bd[:],
                        )
                    ot = outp.tile([C, OB], BF16, tag="ot")
                    rcpv = (
                        rcp_all[:, bass.ds(b * OCH * HEADS, OCH * HEADS)]
                        .rearrange("p (j h) -> p j h", h=HEADS)
                        .broadcast_to([C, OCH, HEADS, HC])
                    )
                    if b in GPB:
                        # normalize off the critical vector path: scalar
                        # casts PSUM->SBUF, gpsimd does the multiply
                        qt = qtmp.tile([C, OB], BF16, tag="qt")
                        nc.scalar.copy(qt[:], att[:])
                        nc.gpsimd.tensor_mul(
                            ot[:].rearrange("p (j h c) -> p j h c", h=HEADS, c=HC),
                            qt[:].rearrange("p (j h c) -> p j h c", h=HEADS, c=HC),
                            rcpv,
                        )
                    else:
                        nc.vector.tensor_mul(
                            ot[:].rearrange("p (j h c) -> p j h c", h=HEADS, c=HC),
                            att[:].rearrange("p (j h c) -> p j h c", h=HEADS, c=HC),
                            rcpv,
                        )
                    eng = nc.scalar if b % 2 == 0 else nc.sync
                    eng.dma_start(
                        out=out_d[:, bass.ds(b * OCH, OCH), :],
                        in_=ot[:].rearrange("p (j c) -> p j c", c=C),
                    )

    nc.compile()
    return nc


def _get_nc():
    if "nc" not in _cache:
        _cache["nc"] = _build()
    return _cache["nc"]


def _consts_np():
    import ml_dtypes

    bf16 = ml_dtypes.bfloat16
    ident = np.eye(C, dtype=np.float32).astype(bf16)
    ind8 = np.zeros((C, HEADS), dtype=np.float32)
    for h in range(HEADS):
        ind8[h * HC:(h + 1) * HC, h] = 1.0
    bd8 = np.zeros((C, C), dtype=np.float32)
    for h in range(HEADS):
        bd8[h * HC:(h + 1) * HC, h * HC:(h + 1) * HC] = 1.0
    return ident, ind8.astype(bf16), bd8.astype(bf16)


def _to_np(a) -> np.ndarray:
    """Materialize to float32 numpy; retry once on a transient bad fetch
    (device-backed arrays have been observed to materialize NaNs once)."""
    out = np.asarray(a, dtype=np.float32)
    if np.isnan(out).any():
        out = np.asarray(a, dtype=np.float32)
    return out


def make_in_maps(x1: np.ndarray, x2: np.ndarray):
    import ml_dtypes

    bf16 = ml_dtypes.bfloat16
    x1 = _to_np(x1).reshape(B, C, N)
    x2 = _to_np(x2).reshape(B, C, N)
    # x1 blocked-transposed: x1t[b, p, j, c] = x1[b, c, j*128 + p]
    x1t = np.ascontiguousarray(
        x1.reshape(B, C, NB, C).transpose(0, 3, 2, 1)
    ).astype(bf16)
    x2b = x2.astype(bf16)
    ident, ind8, bd8 = _consts_np()
    return [
        {"x1t": x1t[i], "x2": x2b[i], "ident": ident, "ind8": ind8, "bd8": bd8}
        for i in range(NCORES)
    ]


def kernel(x1: np.ndarray, x2: np.ndarray) -> np.ndarray:
    from concourse.bass_utils import run_bass_kernel_spmd

    nc = _get_nc()
    in_maps = make_in_maps(x1, x2)
    res = run_bass_kernel_spmd(nc, in_maps, core_ids=list(range(NCORES)))
    outs = []
    for i in range(NCORES):
        o = np.asarray(res.results[i]["out"], dtype=np.float32)  # [128, NB, C]
        outs.append(o.transpose(2, 1, 0).reshape(C, N))          # [C, N]
    return np.stack(outs, axis=0).reshape(B, C, H, W)


# revision 25
# speedup vs baseline: 1.0746x; 1.0347x over previous
"""Trainium2 Bass kernel for cross "efficient attention".

Reference computation (per batch b, head h, with C=128, HEADS=8, hc=16, n=16384):
    k = x2[b].reshape(HEADS, hc, n); v = x1[b].reshape(HEADS, hc, n)
    key_sm   = softmax(k, axis=-1)          # over n
    query_sm = softmax(k, axis=1)           # over hc (head channels)
    context  = key_sm @ v^T                 # (hc, hc)
    out[b,h] = context^T @ query_sm         # (hc, n)

Sharding: data-parallel over batch B=8 across the 8 NeuronCores (no
collectives).  Inputs are cast to bf16 on the host (tolerance is 2e-2;
bf16 end-to-end measures ~4e-3), halving HBM traffic, and x1 is laid
out host-side as [128, N/128, C] blocks so every DMA descriptor is a
contiguous run >= 4 KiB.

Per-core pipeline (N = 16384 in 4 slabs of 4096 = 32 chunks of 128):
  pass 1 (per chunk j):
    MM_t : transpose-mode matmul -> te PSUM bf16 (grouped 8 chunks/tile)
    MM_cs: matmul(lhsT=exp_chunk, rhs=ind8) -> per-slab PSUM f32
           accumulator [128, 32*8] (per-head colsums, transposed layout)
    one wide vector copy te -> eT slab buffer per group (bf16->bf16 2x)
    MM_ctx (one slab lag, interleaved per group): ctx += eT^T @ vT
  per slab: one reciprocal_approx_fast [128,256] PSUM->SBUF (rcp_all)
  bd = (ctx / rowsum) * blockdiag_mask  (bf16 [C,C])
  pass 2 (per 2048 block, chunk j): MM_att: matmul(lhsT=exp_chunk,
    rhs=bd) -> attT PSUM f32 [128, 2048]; one vector tensor_mul per
    block with the per-head reciprocals broadcast via a stride-0 AP ->
    bf16 out tile -> DMA out (ACT HWDGE ring).
Output leaves the device transposed ([128, N/128, C] blocks); the host
reassembles [C, H, W].
"""

import numpy as np
from contextlib import ExitStack

B, C, H, W = 8, 128, 128, 128
N = H * W                 # 16384
HEADS, HC = 8, 16
NCORES = 8
# pass-1 slab widths: wide while DMA-bound, tapered at the end so the
# final slab's exp->transpose->copy->ctx chain (which gates pass 2) is
# short.
SLABS = [1024, 2048, 4096, 4096, 2048, 1024, 1024, 512, 512]
NSLAB = len(SLABS)
assert sum(SLABS) == N
NB = N // C               # 128 chunk-blocks total
GRP = 8                   # transpose chunks batched per PSUM group tile
OB = 2048                 # pass-2 output block width
NOB = N // OB             # 8
OCH = OB // C             # chunks per output block = 16

_cache: dict = {}


def _build():
    import concourse.bass as bass
    import concourse.tile as tile
    from concourse import bacc, mybir

    FP32 = mybir.dt.float32
    BF16 = mybir.dt.bfloat16
    AF = mybir.ActivationFunctionType

    nc = bacc.Bacc("TRN2", target_bir_lowering=False, debug=False)

    x1t_d = nc.dram_tensor("x1t", [C, NB, C], BF16, kind="ExternalInput")
    x2_d = nc.dram_tensor("x2", [C, N], BF16, kind="ExternalInput")
    id_d = nc.dram_tensor("ident", [C, C], BF16, kind="ExternalInput")
    ind8_d = nc.dram_tensor("ind8", [C, HEADS], BF16, kind="ExternalInput")
    bd8_d = nc.dram_tensor("bd8", [C, C], BF16, kind="ExternalInput")
    out_d = nc.dram_tensor("out", [C, NB, C], BF16, kind="ExternalOutput")

    with tile.TileContext(nc) as tc:
        with ExitStack() as ctx:
            persist = ctx.enter_context(tc.tile_pool(name="persist", bufs=1))
            x2ld = ctx.enter_context(tc.tile_pool(name="x2ld", bufs=6))
            vTp = ctx.enter_context(tc.tile_pool(name="vTp", bufs=6))
            eTp = ctx.enter_context(tc.tile_pool(name="eTp", bufs=3))
            outp = ctx.enter_context(tc.tile_pool(name="outp", bufs=4))
            smalls = ctx.enter_context(tc.tile_pool(name="smalls", bufs=1))

            # one exp tile per slab: a single big tile would serialize the
            # pipeline on write-after-read hazards (slab i+1's exp write
            # waits for slab i's transpose reads at tile granularity)
            exp_tiles = [
                persist.tile([C, SW], BF16, tag=f"exp{i}", name=f"exp{i}")
                for i, SW in enumerate(SLABS)
            ]
            rcp_all = persist.tile([C, NB * HEADS], FP32, tag="rcp_all")
            rs_acc = smalls.tile([C, NSLAB], FP32, tag="rs_acc")
            ident = smalls.tile([C, C], BF16, tag="ident")
            ind8 = smalls.tile([C, HEADS], BF16, tag="ind8")
            bd8 = smalls.tile([C, C], BF16, tag="bd8")
            bd = smalls.tile([C, C], BF16, tag="bd")

            with tc.tile_pool(name="psctx", bufs=1, space="PSUM") as ps_ctx, \
                 tc.tile_pool(name="pstre", bufs=4, space="PSUM") as ps_te, \
                 tc.tile_pool(name="pscs", bufs=2, space="PSUM") as ps_cs:
                ctx_ps = ps_ctx.tile([C, C], FP32, tag="ctx")

                mm_idx = 0
                pending = []   # (eT_ap, vT_ap) per not-yet-contracted chunk

                def emit_ctx(k):
                    # emit ctx matmuls for the first k pending chunks
                    nonlocal mm_idx
                    for eTc, vTc in pending[:k]:
                        nc.tensor.matmul(
                            ctx_ps[:], eTc, vTc,
                            start=(mm_idx == 0),
                            stop=(mm_idx == NB - 1),
                        )
                        mm_idx += 1
                    del pending[:k]

                off = 0
                chunk_aps = []   # global chunk index -> exp chunk AP
                for i, SW in enumerate(SLABS):
                    nch = SW // C
                    ngrp = (nch + GRP - 1) // GRP
                    x2t = x2ld.tile([C, SW], BF16, tag="x2t")
                    nc.sync.dma_start(out=x2t[:], in_=x2_d[:, bass.ds(off, SW)])
                    vT = vTp.tile([C, SW], BF16, tag="vT")
                    nc.sync.dma_start(
                        out=vT[:].rearrange("p (j c) -> p j c", c=C),
                        in_=x1t_d[:, bass.ds(off // C, nch), :],
                    )
                    if i == 0:
                        # constants ride the ACT ring so they don't delay
                        # the slab loads on the SP ring
                        nc.scalar.dma_start(out=ident[:], in_=id_d[:])
                        nc.scalar.dma_start(out=ind8[:], in_=ind8_d[:])
                        nc.scalar.dma_start(out=bd8[:], in_=bd8_d[:])

                    exp_sl = exp_tiles[i]
                    nc.scalar.activation(
                        exp_sl[:], x2t[:], AF.Exp,
                        accum_out=rs_acc[:, i:i + 1],
                    )

                    # per-chunk transpose (bf16, grouped PSUM tiles) +
                    # colsum (f32 accum tile); one wide copy per group;
                    # pending (prev-slab) ctx matmuls interleaved per group
                    eT = eTp.tile([C, SW], BF16, tag="eT")
                    eTv = eT[:].rearrange("p (j c) -> p j c", c=C)
                    vTv = vT[:].rearrange("p (j c) -> p j c", c=C)
                    cs_ps = ps_cs.tile([C, nch * HEADS], FP32, tag="cs")
                    for g in range(ngrp):
                        gsz = min(GRP, nch - g * GRP)
                        te = ps_te.tile([C, gsz * C], BF16, tag="te")
                        fresh = []
                        for jj in range(gsz):
                            j = g * GRP + jj
                            e_chunk = exp_sl[:, bass.ds(j * C, C)]
                            chunk_aps.append(e_chunk)
                            nc.tensor.transpose(
                                te[:, bass.ds(jj * C, C)], e_chunk, ident[:]
                            )
                            nc.tensor.matmul(
                                cs_ps[:, bass.ds(j * HEADS, HEADS)],
                                e_chunk, ind8[:],
                            )
                            fresh.append((eTv[:, j, :], vTv[:, j, :]))
                        nc.vector.tensor_copy(
                            eT[:, bass.ds(g * GRP * C, gsz * C)], te[:]
                        )
                        # ctx matmuls lag one group behind the copies
                        emit_ctx(len(pending))
                        pending.extend(fresh)

                    # per-slab: one reciprocal over the colsum accumulator
                    nc.vector.reciprocal_approx_fast(
                        out=rcp_all[:, bass.ds(off // C * HEADS, nch * HEADS)],
                        in_=cs_ps[:],
                    )
                    off += SW
                emit_ctx(len(pending))

                # ---- block-diagonal context weights ----
                rowsum = smalls.tile([C, 1], FP32, tag="rowsum")
                nc.vector.tensor_reduce(
                    rowsum[:], rs_acc[:], mybir.AxisListType.X, mybir.AluOpType.add
                )
                rs_rcp = smalls.tile([C, 1], FP32, tag="rs_rcp")
                nc.vector.reciprocal(rs_rcp[:], rowsum[:])
                scaled = smalls.tile([C, C], BF16, tag="scaled")
                nc.vector.tensor_scalar(
                    scaled[:], ctx_ps[:], rs_rcp[:, 0:1], None, mybir.AluOpType.mult
                )
                nc.vector.tensor_mul(bd[:], scaled[:], bd8[:])

            # ---- pass 2: attended (transposed), normalize, store ----
            with tc.tile_pool(name="psatt", bufs=2, space="PSUM") as ps_att:
                for b in range(NOB):
                    att = ps_att.tile([C, OB], FP32, tag="att")
                    for j in range(OCH):
                        nc.tensor.matmul(
                            att[:, bass.ds(j * C, C)],
                            chunk_aps[b * OCH + j],
                            bd[:],
                        )
                    ot = outp.tile([C, OB], BF16, tag="ot")
                    nc.vector.tensor_mul(
                        ot[:].rearrange("p (j h c) -> p j h c", h=HEADS, c=HC),
                        att[:].rearrange("p (j h c) -> p j h c", h=HEADS, c=HC),
                        rcp_all[:, bass.ds(b * OCH * HEADS, OCH * HEADS)]
                        .rearrange("p (j h) -> p j h", h=HEADS)
                        .broadcast_to([C, OCH, HEADS, HC]),
                    )
                    eng = nc.scalar if b % 2 == 0 else nc.sync
                    eng.dma_start(
                        out=out_d[:, bass.ds(b * OCH, OCH), :],
                        in_=ot[:].rearrange("p (j c) -> p j c", c=C),
                    )

    nc.compile()
    return nc


def _get_nc():
    if "nc" not in _cache:
        _cache["nc"] = _build()
    return _cache["nc"]


def _consts_np():
    import ml_dtypes

    bf16 = ml_dtypes.bfloat16
    ident = np.eye(C, dtype=np.float32).astype(bf16)
    ind8 = np.zeros((C, HEADS), dtype=np.float32)
    for h in range(HEADS):
        ind8[h * HC:(h + 1) * HC, h] = 1.0
    bd8 = np.zeros((C, C), dtype=np.float32)
    for h in range(HEADS):
        bd8[h * HC:(h + 1) * HC, h * HC:(h + 1) * HC] = 1.0
    return ident, ind8.astype(bf16), bd8.astype(bf16)


def _to_np(a) -> np.ndarray:
    """Materialize to float32 numpy; retry once on a transient bad fetch
    (device-backed arrays have been observed to materialize NaNs once)."""
    out = np.asarray(a, dtype=np.float32)
    if np.isnan(out).any():
        out = np.asarray(a, dtype=np.float32)
    return out


def make_in_maps(x1: np.ndarray, x2: np.ndarray):
    import ml_dtypes

    bf16 = ml_dtypes.bfloat16
    x1 = _to_np(x1).reshape(B, C, N)
    x2 = _to_np(x2).reshape(B, C, N)
    # x1 blocked-transposed: x1t[b, p, j, c] = x1[b, c, j*128 + p]
    x1t = np.ascontiguousarray(
        x1.reshape(B, C, NB, C).transpose(0, 3, 2, 1)
    ).astype(bf16)
    x2b = x2.astype(bf16)
    ident, ind8, bd8 = _consts_np()
    return [
        {"x1t": x1t[i], "x2": x2b[i], "ident": ident, "ind8": ind8, "bd8": bd8}
        for i in range(NCORES)
    ]


def kernel(x1: np.ndarray, x2: np.ndarray) -> np.ndarray:
    from concourse.bass_utils import run_bass_kernel_spmd

    nc = _get_nc()
    in_maps = make_in_maps(x1, x2)
    res = run_bass_kernel_spmd(nc, in_maps, core_ids=list(range(NCORES)))
    outs = []
    for i in range(NCORES):
        o = np.asarray(res.results[i]["out"], dtype=np.float32)  # [128, NB, C]
        outs.append(o.transpose(2, 1, 0).reshape(C, N))          # [C, N]
    return np.stack(outs, axis=0).reshape(B, C, H, W)


# revision 26
# speedup vs baseline: 1.0851x; 1.0098x over previous
"""Trainium2 Bass kernel for cross "efficient attention".

Reference computation (per batch b, head h, with C=128, HEADS=8, hc=16, n=16384):
    k = x2[b].reshape(HEADS, hc, n); v = x1[b].reshape(HEADS, hc, n)
    key_sm   = softmax(k, axis=-1)          # over n
    query_sm = softmax(k, axis=1)           # over hc (head channels)
    context  = key_sm @ v^T                 # (hc, hc)
    out[b,h] = context^T @ query_sm         # (hc, n)

Sharding: data-parallel over batch B=8 across the 8 NeuronCores (no
collectives).  Inputs are cast to bf16 on the host (tolerance is 2e-2;
bf16 end-to-end measures ~4e-3), halving HBM traffic, and x1 is laid
out host-side as [128, N/128, C] blocks so every DMA descriptor is a
contiguous run >= 4 KiB.

Per-core pipeline (N = 16384 in 4 slabs of 4096 = 32 chunks of 128):
  pass 1 (per chunk j):
    MM_t : transpose-mode matmul -> te PSUM bf16 (grouped 8 chunks/tile)
    MM_cs: matmul(lhsT=exp_chunk, rhs=ind8) -> per-slab PSUM f32
           accumulator [128, 32*8] (per-head colsums, transposed layout)
    one wide vector copy te -> eT slab buffer per group (bf16->bf16 2x)
    MM_ctx (one slab lag, interleaved per group): ctx += eT^T @ vT
  per slab: one reciprocal_approx_fast [128,256] PSUM->SBUF (rcp_all)
  bd = (ctx / rowsum) * blockdiag_mask  (bf16 [C,C])
  pass 2 (per 2048 block, chunk j): MM_att: matmul(lhsT=exp_chunk,
    rhs=bd) -> attT PSUM f32 [128, 2048]; one vector tensor_mul per
    block with the per-head reciprocals broadcast via a stride-0 AP ->
    bf16 out tile -> DMA out (ACT HWDGE ring).
Output leaves the device transposed ([128, N/128, C] blocks); the host
reassembles [C, H, W].
"""

import numpy as np
from contextlib import ExitStack

B, C, H, W = 8, 128, 128, 128
N = H * W                 # 16384
HEADS, HC = 8, 16
NCORES = 8
# pass-1 slab widths: wide while DMA-bound, tapered at the end so the
# final slab's exp->transpose->copy->ctx chain (which gates pass 2) is
# short.
SLABS = [1024, 2048, 4096, 4096, 2048, 1024, 1024, 512, 512]
NSLAB = len(SLABS)
assert sum(SLABS) == N
NB = N // C               # 128 chunk-blocks total
GRP = 8                   # transpose chunks batched per PSUM group tile
OB = 2048                 # pass-2 output block width
NOB = N // OB             # 8
OCH = OB // C             # chunks per output block = 16

_cache: dict = {}


def _build():
    import concourse.bass as bass
    import concourse.tile as tile
    from concourse import bacc, mybir

    FP32 = mybir.dt.float32
    BF16 = mybir.dt.bfloat16
    AF = mybir.ActivationFunctionType

    nc = bacc.Bacc("TRN2", target_bir_lowering=False, debug=False)

    x1t_d = nc.dram_tensor("x1t", [C, NB, C], BF16, kind="ExternalInput")
    x2_d = nc.dram_tensor("x2", [C, N], BF16, kind="ExternalInput")
    id_d = nc.dram_tensor("ident", [C, C], BF16, kind="ExternalInput")
    ind8_d = nc.dram_tensor("ind8", [C, HEADS], BF16, kind="ExternalInput")
    bd8_d = nc.dram_tensor("bd8", [C, C], BF16, kind="ExternalInput")
    out_d = nc.dram_tensor("out", [C, NB, C], BF16, kind="ExternalOutput")

    with tile.TileContext(nc) as tc:
        with ExitStack() as ctx:
            persist = ctx.enter_context(tc.tile_pool(name="persist", bufs=1))
            x2ld = ctx.enter_context(tc.tile_pool(name="x2ld", bufs=6))
            vTp = ctx.enter_context(tc.tile_pool(name="vTp", bufs=6))
            eTp = ctx.enter_context(tc.tile_pool(name="eTp", bufs=3))
            outp = ctx.enter_context(tc.tile_pool(name="outp", bufs=4))
            smalls = ctx.enter_context(tc.tile_pool(name="smalls", bufs=1))

            # one exp tile per slab: a single big tile would serialize the
            # pipeline on write-after-read hazards (slab i+1's exp write
            # waits for slab i's transpose reads at tile granularity)
            exp_tiles = [
                persist.tile([C, SW], BF16, tag=f"exp{i}", name=f"exp{i}")
                for i, SW in enumerate(SLABS)
            ]
            rcp_all = persist.tile([C, NB * HEADS], FP32, tag="rcp_all")
            rs_acc = smalls.tile([C, NSLAB], FP32, tag="rs_acc")
            ident = smalls.tile([C, C], BF16, tag="ident")
            ind8 = smalls.tile([C, HEADS], BF16, tag="ind8")
            bd8 = smalls.tile([C, C], BF16, tag="bd8")
            bd = smalls.tile([C, C], BF16, tag="bd")

            with tc.tile_pool(name="psctx", bufs=1, space="PSUM") as ps_ctx, \
                 tc.tile_pool(name="pstre", bufs=4, space="PSUM") as ps_te, \
                 tc.tile_pool(name="pscs", bufs=2, space="PSUM") as ps_cs:
                ctx_ps = ps_ctx.tile([C, C], FP32, tag="ctx")

                mm_idx = 0
                pending = []   # (eT_ap, vT_ap) per not-yet-contracted chunk

                def emit_ctx(k):
                    # emit ctx matmuls for the first k pending chunks
                    nonlocal mm_idx
                    for eTc, vTc in pending[:k]:
                        nc.tensor.matmul(
                            ctx_ps[:], eTc, vTc,
                            start=(mm_idx == 0),
                            stop=(mm_idx == NB - 1),
                        )
                        mm_idx += 1
                    del pending[:k]

                off = 0
                chunk_aps = []   # global chunk index -> exp chunk AP
                for i, SW in enumerate(SLABS):
                    nch = SW // C
                    ngrp = (nch + GRP - 1) // GRP
                    x2t = x2ld.tile([C, SW], BF16, tag="x2t")
                    # slab 0's x2 rides SWDGE: the gpsimd queue exits the
                    # start preamble ~3us before the SP HWDGE ring, and
                    # exp(0) (the pass-1 pipeline head) only needs x2
                    ldeng = nc.gpsimd if i == 0 else nc.sync
                    ldeng.dma_start(out=x2t[:], in_=x2_d[:, bass.ds(off, SW)])
                    vT = vTp.tile([C, SW], BF16, tag="vT")
                    nc.sync.dma_start(
                        out=vT[:].rearrange("p (j c) -> p j c", c=C),
                        in_=x1t_d[:, bass.ds(off // C, nch), :],
                    )
                    if i == 0:
                        # constants ride the ACT ring so they don't delay
                        # the slab loads on the SP ring
                        nc.scalar.dma_start(out=ident[:], in_=id_d[:])
                        nc.scalar.dma_start(out=ind8[:], in_=ind8_d[:])
                        nc.scalar.dma_start(out=bd8[:], in_=bd8_d[:])

                    exp_sl = exp_tiles[i]
                    nc.scalar.activation(
                        exp_sl[:], x2t[:], AF.Exp,
                        accum_out=rs_acc[:, i:i + 1],
                    )

                    # per-chunk transpose (bf16, grouped PSUM tiles) +
                    # colsum (f32 accum tile); one wide copy per group;
                    # pending (prev-slab) ctx matmuls interleaved per group
                    eT = eTp.tile([C, SW], BF16, tag="eT")
                    eTv = eT[:].rearrange("p (j c) -> p j c", c=C)
                    vTv = vT[:].rearrange("p (j c) -> p j c", c=C)
                    cs_ps = ps_cs.tile([C, nch * HEADS], FP32, tag="cs")
                    for g in range(ngrp):
                        gsz = min(GRP, nch - g * GRP)
                        te = ps_te.tile([C, gsz * C], BF16, tag="te")
                        fresh = []
                        for jj in range(gsz):
                            j = g * GRP + jj
                            e_chunk = exp_sl[:, bass.ds(j * C, C)]
                            chunk_aps.append(e_chunk)
                            nc.tensor.transpose(
                                te[:, bass.ds(jj * C, C)], e_chunk, ident[:]
                            )
                            nc.tensor.matmul(
                                cs_ps[:, bass.ds(j * HEADS, HEADS)],
                                e_chunk, ind8[:],
                            )
                            fresh.append((eTv[:, j, :], vTv[:, j, :]))
                        nc.vector.tensor_copy(
                            eT[:, bass.ds(g * GRP * C, gsz * C)], te[:]
                        )
                        # ctx matmuls lag one group behind the copies
                        emit_ctx(len(pending))
                        pending.extend(fresh)

                    # per-slab: one reciprocal over the colsum accumulator
                    nc.vector.reciprocal_approx_fast(
                        out=rcp_all[:, bass.ds(off // C * HEADS, nch * HEADS)],
                        in_=cs_ps[:],
                    )
                    off += SW
                emit_ctx(len(pending))

                # ---- block-diagonal context weights ----
                rowsum = smalls.tile([C, 1], FP32, tag="rowsum")
                nc.vector.tensor_reduce(
                    rowsum[:], rs_acc[:], mybir.AxisListType.X, mybir.AluOpType.add
                )
                rs_rcp = smalls.tile([C, 1], FP32, tag="rs_rcp")
                nc.vector.reciprocal(rs_rcp[:], rowsum[:])
                scaled = smalls.tile([C, C], BF16, tag="scaled")
                nc.vector.tensor_scalar(
                    scaled[:], ctx_ps[:], rs_rcp[:, 0:1], None, mybir.AluOpType.mult
                )
                nc.vector.tensor_mul(bd[:], scaled[:], bd8[:])

            # ---- pass 2: attended (transposed), normalize, store ----
            with tc.tile_pool(name="psatt", bufs=2, space="PSUM") as ps_att:
                for b in range(NOB):
                    att = ps_att.tile([C, OB], FP32, tag="att")
                    for j in range(OCH):
                        nc.tensor.matmul(
                            att[:, bass.ds(j * C, C)],
                            chunk_aps[b * OCH + j],
                            bd[:],
                        )
                    ot = outp.tile([C, OB], BF16, tag="ot")
                    nc.vector.tensor_mul(
                        ot[:].rearrange("p (j h c) -> p j h c", h=HEADS, c=HC),
                        att[:].rearrange("p (j h c) -> p j h c", h=HEADS, c=HC),
                        rcp_all[:, bass.ds(b * OCH * HEADS, OCH * HEADS)]
                        .rearrange("p (j h) -> p j h", h=HEADS)
                        .broadcast_to([C, OCH, HEADS, HC]),
                    )
                    eng = nc.scalar if b % 2 == 0 else nc.sync
                    eng.dma_start(
                        out=out_d[:, bass.ds(b * OCH, OCH), :],
                        in_=ot[:].rearrange("p (j c) -> p j c", c=C),
                    )

    nc.compile()
    return nc


def _get_nc():
    if "nc" not in _cache:
        _cache["nc"] = _build()
    return _cache["nc"]


def _consts_np():
    import ml_dtypes

    bf16 = ml_dtypes.bfloat16
    ident = np.eye(C, dtype=np.float32).astype(bf16)
    ind8 = np.zeros((C, HEADS), dtype=np.float32)
    for h in range(HEADS):
        ind8[h * HC:(h + 1) * HC, h] = 1.0
    bd8 = np.zeros((C, C), dtype=np.float32)
    for h in range(HEADS):
        bd8[h * HC:(h + 1) * HC, h * HC:(h + 1) * HC] = 1.0
    return ident, ind8.astype(bf16), bd8.astype(bf16)


def _to_np(a) -> np.ndarray:
    """Materialize to float32 numpy; retry once on a transient bad fetch
    (device-backed arrays have been observed to materialize NaNs once)."""
    out = np.asarray(a, dtype=np.float32)
    if np.isnan(out).any():
        out = np.asarray(a, dtype=np.float32)
    return out


def make_in_maps(x1: np.ndarray, x2: np.ndarray):
    import ml_dtypes

    bf16 = ml_dtypes.bfloat16
    x1 = _to_np(x1).reshape(B, C, N)
    x2 = _to_np(x2).reshape(B, C, N)
    # x1 blocked-transposed: x1t[b, p, j, c] = x1[b, c, j*128 + p]
    x1t = np.ascontiguousarray(
        x1.reshape(B, C, NB, C).transpose(0, 3, 2, 1)
    ).astype(bf16)
    x2b = x2.astype(bf16)
    ident, ind8, bd8 = _consts_np()
    return [
        {"x1t": x1t[i], "x2": x2b[i], "ident": ident, "ind8": ind8, "bd8": bd8}
        for i in range(NCORES)
    ]


def kernel(x1: np.ndarray, x2: np.ndarray) -> np.ndarray:
    from concourse.bass_utils import run_bass_kernel_spmd

    nc = _get_nc()
    in_maps = make_in_maps(x1, x2)
    res = run_bass_kernel_spmd(nc, in_maps, core_ids=list(range(NCORES)))
    outs = []
    for i in range(NCORES):
        o = np.asarray(res.results[i]["out"], dtype=np.float32)  # [128, NB, C]
        outs.append(o.transpose(2, 1, 0).reshape(C, N))          # [C, N]
    return np.stack(outs, axis=0).reshape(B, C, H, W)
